# revision 17
# baseline (speedup 1.0000x reference)
"""MoE low-rank adapters (top-1 routing) Trainium2 kernel.

Math (reference):
  xf = x.reshape(N, D)                 N=8192, D=2048, E=8, R=64
  logits = xf @ Wg.T                   [N, E]
  prob = softmax(logits); gate = argmax(prob); prob_sel = max(prob)
  h = xf @ A[e].T for all e            [N, E*R]
  y = (h * onehot(gate)) @ Bwt         [N, D]
  y *= SCALING * prob_sel

Distribution: data-parallel over tokens, 8 cores x 1024 tokens.

Design (v9, fp16 end-to-end):
- Everything lives in fp16: x (host-cast; gating argmax verified safe in
  fp16 with ~40x margin on the worst top-2 logit gap), A, Bw, Wg, y, and
  the mask staging. Halves x DMA vs f32 and removes the on-chip
  f32->bf16 cast stage entirely: h matmuls read the same x tiles as
  gating. Total DMA ~12.3MB/core vs ~19MB in the f32r/bf16 version.
- Pair-0 front is k-major: per k-chunk, one gating matmul plus the four
  h-bank matmuls consume the x/A granule that just streamed in. PE goes
  continuously busy right after a short warmup, so the HAM clock ramps
  to 2.4GHz early and stays there (gaps are what hold it at 1.2GHz).
- PE order: warmup, {g0,h0}x16, [epilogue0], g1, y0[0..14], h1,
  y0[15] (deferred chain fills the hm1 wait), y1. Softmax/mask epilogue
  ops slot between chains; the pair-1 mask DRAM broadcast rides gpsimd
  while y0 runs.
- PSUM: 4 h banks + lg + tr + 2 y banks = 8. y chains rotate over three
  slots (y0, y1, + lg during pair-0 emit / + h0 bank during pair-1
  emit) so bank-release copies stay off the next chain's critical path.
- DMA: 256KB-512KB pieces round-robined over the three queues
  (sync/scalar HWDGE, gpsimd SWDGE) in priority order: wg, {x0,A}
  interleaved k-granules, x1, mask0 staging, B per-o pieces, y-out per
  tok-chunk. Final tok-chunk drains per-o across all three queues.
"""

import sys

for _p in ("/opt/trn_rl_repo",):
    if _p not in sys.path:
        sys.path.insert(0, _p)

import numpy as np

import concourse.bass as bass
import concourse.bacc as bacc
import concourse.mybir as mybir
import concourse.tile as tile
from concourse import bass_utils
from concourse.masks import make_identity

f32 = mybir.dt.float32
f16 = mybir.dt.float16
bf16 = mybir.dt.bfloat16

B, S, D, R, E = 4, 2048, 2048, 64, 8
N = B * S                    # 8192 tokens
NCORES = 8
NTOK = N // NCORES           # 1024 tokens per core
SCALING = 64.0 / 16.0
ER = E * R                   # 512
KD = D // 128                # 16 d-chunks
PBLK = 512                   # tokens per pair-block
NPAIR = NTOK // PBLK         # 2
ERCH = ER // 128             # er chunks (4)
NT = PBLK // 128             # tok-chunks per pair (4)
NOCH = D // 512              # output chunks of 512 (4)
KG = 2                       # k-chunks per DMA granule

_CACHE = {}


def _build():
    if "nc" in _CACHE:
        return _CACHE["nc"]
    nc = bacc.Bacc("TRN2", target_bir_lowering=False, debug=False,
                   num_devices=NCORES)
    xt = nc.dram_tensor("xt", [128, KD, NTOK], f16, kind="ExternalInput")
    af = nc.dram_tensor("af", [128, KD, ER], f16, kind="ExternalInput")
    bw = nc.dram_tensor("bw", [128, NOCH, ERCH, 512], f16, kind="ExternalInput")
    wg = nc.dram_tensor("wg", [128, KD, E], f16, kind="ExternalInput")
    yo = nc.dram_tensor("yo", [NPAIR, NT, 128, D], f16, kind="ExternalOutput")
    mstage = nc.dram_tensor("mstage", [NPAIR, NT * E, 128], f16,
                            kind="Internal")

    with tile.TileContext(nc) as tc:
        import contextlib
        ctx = contextlib.ExitStack()
        with ctx:
            singles = ctx.enter_context(tc.tile_pool(name="singles", bufs=1))
            hpool = ctx.enter_context(tc.tile_pool(name="hpool", bufs=2))
            mpool = ctx.enter_context(tc.tile_pool(name="mpool", bufs=2))
            spool = ctx.enter_context(tc.tile_pool(name="spool", bufs=2))
            ypool = ctx.enter_context(tc.tile_pool(name="ypool", bufs=4))
            ps_h = ctx.enter_context(tc.tile_pool(name="ps_h", bufs=1, space="PSUM"))
            ps_lg = ctx.enter_context(tc.tile_pool(name="ps_lg", bufs=1, space="PSUM"))
            ps_tr = ctx.enter_context(tc.tile_pool(name="ps_tr", bufs=1, space="PSUM"))
            ps_y = ctx.enter_context(tc.tile_pool(name="ps_y", bufs=1, space="PSUM"))

            qs = (nc.sync, nc.scalar, nc.gpsimd)

            # ---- identities + gate weights ----
            ident = singles.tile([128, 128], f32)
            make_identity(nc, ident)
            identb = singles.tile([128, 128], bf16)
            make_identity(nc, identb)
            wg_sb = singles.tile([128, KD, E], f16)
            nc.sync.dma_start(out=wg_sb, in_=wg.ap())

            # ---- big fp16 SBUF tiles; DMAs land in slices ----
            xs = singles.tile([128, KD, NTOK], f16)
            ab = singles.tile([128, KD, ER], f16)
            bwo = singles.tile([128, NOCH, ERCH, 512], f16)

            # phase 0: full-token-width x granules (2KB runs), then A
            qi = [0]

            def q():
                e = qs[qi[0] % 3]
                qi[0] += 1
                return e

            for g in range(KD // KG):
                k0 = KG * g
                q().dma_start(out=xs[:, k0:k0 + KG, :],
                              in_=xt.ap()[:, k0:k0 + KG, :])
            for g in range(KD // KG):
                k0 = KG * g
                q().dma_start(out=ab[:, k0:k0 + KG, :],
                              in_=af.ap()[:, k0:k0 + KG, :])

            def issue_b():
                # per-o pieces (4KB runs): y chain (t,o) reads bwo[:, o, i, :]
                for o in range(NOCH):
                    eng = (nc.scalar, nc.gpsimd, nc.scalar, nc.gpsimd)[o]
                    eng.dma_start(out=bwo[:, o], in_=bw.ap()[:, o])

            # ---- PE warm-up while the first granules stream in ----
            wup = ps_y.tile([128, 512], f32, tag="y0", name="warmup_ps")

            def dummy(n):
                # clock-keeper: PE stays busy through DMA-paced stretches so
                # the HAM clock ramps once and stays at 2.4GHz
                for _ in range(n):
                    nc.tensor.matmul(wup[:, 0:128], identb, identb,
                                     start=True, stop=True)

            dummy(12)

            def gating_and_h0():
                # k-major: per k, gating + all four h banks consume the
                # granule that just landed -> PE continuously busy from the
                # start, HAM clock ramps once
                lg = ps_lg.tile([128, PBLK], f32, tag="lg", name="lg0")
                hps = [ps_h.tile([128, PBLK], f32, tag=f"h{i}",
                                 name=f"h0_{i}") for i in range(ERCH)]
                for k in range(KD):
                    nc.tensor.matmul(
                        lg[0:E, :], wg_sb[:, k, :], xs[:, k, 0:PBLK],
                        start=(k == 0), stop=(k == KD - 1))
                    for i in range(ERCH):
                        nc.tensor.matmul(
                            hps[i], ab[:, k, 128 * i:128 * i + 128],
                            xs[:, k, 0:PBLK],
                            start=(k == 0), stop=(k == KD - 1))
                return lg, hps

            def gating1():
                lg = ps_lg.tile([128, PBLK], f32, tag="lg", name="lg1")
                for k in range(KD):
                    nc.tensor.matmul(
                        lg[0:E, :], wg_sb[:, k, :], xs[:, k, PBLK:NTOK],
                        start=(k == 0), stop=(k == KD - 1))
                return lg

            def h1_block():
                # pair-1 h into the four non-h PSUM banks (y0/y1/lg/tr) so
                # it runs back-to-back after g1 with no mask dependency
                hps = [ps_y.tile([128, PBLK], f32, tag="y0", name="h1_0"),
                       ps_y.tile([128, PBLK], f32, tag="y1", name="h1_1"),
                       ps_lg.tile([128, PBLK], f32, tag="lg", name="h1_2"),
                       ps_tr.tile([128, PBLK], f32, tag="tr", name="h1_3")]
                for k in range(KD):
                    for i in range(ERCH):
                        nc.tensor.matmul(
                            hps[i], ab[:, k, 128 * i:128 * i + 128],
                            xs[:, k, PBLK:NTOK],
                            start=(k == 0), stop=(k == KD - 1))
                return hps

            def lg_copy(lg):
                lg_sb = spool.tile([E, PBLK], f32, tag="lg_sb")
                nc.scalar.copy(lg_sb, lg[0:E, :])
                return lg_sb

            def tr_logits(pair, lg_sb):
                # pair 1 can't use the tr bank (h1 holds it until hm1, which
                # transitively needs this transpose) -> use h3, free by then
                pool, tag = (ps_tr, "tr") if pair == 0 else (ps_h, "h3")
                eptr = pool.tile([128, 512], f32, tag=tag, name=f"tr{pair}")
                for t in range(NT):
                    nc.tensor.transpose(
                        eptr[:, 8 * t:8 * t + 8],
                        lg_sb[:, 128 * t:128 * t + 128], ident[0:E, 0:E])
                return eptr

            def softmax_mask(pair, eptr):
                mxs = []
                for t in range(NT):
                    mx = spool.tile([128, 1], f32, tag=f"mx{t}")
                    nc.vector.reduce_max(out=mx, in_=eptr[:, 8 * t:8 * t + 8],
                                         axis=mybir.AxisListType.X)
                    mxs.append(mx)
                negs = []
                for t in range(NT):
                    negmx = spool.tile([128, 1], f32, tag=f"negmx{t}")
                    nc.vector.tensor_scalar_mul(negmx, mxs[t], -1.0)
                    negs.append(negmx)
                ses = []
                for t in range(NT):
                    es = spool.tile([128, 8], f32, tag=f"es{t}")
                    se = spool.tile([128, 1], f32, tag=f"se{t}")
                    nc.scalar.activation(out=es, in_=eptr[:, 8 * t:8 * t + 8],
                                         func=mybir.ActivationFunctionType.Exp,
                                         bias=negs[t], scale=1.0, accum_out=se)
                    ses.append(se)
                rcps = []
                for t in range(NT):
                    rcp = spool.tile([128, 1], f32, tag=f"rcp{t}")
                    nc.vector.reciprocal(rcp, ses[t])
                    rcps.append(rcp)
                mval4 = spool.tile([128, NT, 8], f32, tag="mval4")
                for t in range(NT):
                    nc.vector.tensor_scalar(
                        out=mval4[:, t, :], in0=eptr[:, 8 * t:8 * t + 8],
                        scalar1=mxs[t], scalar2=rcps[t],
                        op0=mybir.AluOpType.is_equal,
                        op1=mybir.AluOpType.mult)
                return mval4

            def mask_transpose(pair, eptr, mval4):
                # one fused transpose [128 tok, (t e)=32] -> [32, 128]
                nc.tensor.transpose(eptr[0:32, 128:256],
                                    mval4.rearrange("p t e -> p (t e)"), ident)
                mvT4 = mpool.tile([32, 128], f16, tag="mvT4")
                nc.scalar.copy(mvT4, eptr[0:32, 128:256])
                return mvT4

            def mask_expand(pair, mvT4, eng):
                # stage to DRAM, broadcast over the 64 ranks of each expert
                eng.dma_start(out=mstage.ap()[pair], in_=mvT4)
                mexp = []
                for i in range(ERCH):
                    me = mpool.tile([128, PBLK], f16, tag=f"me{i}")
                    mexp.append(me)
                    for half_e in range(2):
                        srcap = bass.AP(
                            tensor=mstage,
                            offset=(pair * (NT * E * 128)
                                    + (2 * i + half_e) * 128),
                            ap=[[0, 64], [E * 128, NT], [1, 128]],
                        )
                        eng.dma_start(
                            out=me[64 * half_e:64 * half_e + 64, :]
                            .rearrange("p (t n) -> p t n", t=NT),
                            in_=srcap)
                return mexp

            def hm_mask(pair, hps, mexp):
                hm = []
                for i in range(ERCH):
                    t_ = hpool.tile([128, PBLK], f16, tag=f"hm{i}",
                                    name=f"hm{pair}_{i}")
                    nc.vector.tensor_mul(t_, hps[i], mexp[i])
                    hm.append(t_)
                return hm

            yrot = [0]

            def ypsum(name, nslots):
                # y chains rotate over h banks (free after the hm muls);
                # h3 is reserved for pair-1's epilogue during the y0 phase
                tag = f"h{yrot[0] % nslots}"
                yrot[0] += 1
                return ps_h.tile([128, 512], f32, tag=tag, name=name)

            ysbs = {}

            def y_chain(pair, t, o, hmT, nslots):
                key = (pair, t)
                if key not in ysbs:
                    ysbs[key] = ypool.tile([128, D], f16, tag="ysb",
                                           name=f"ysb{pair}_{t}")
                ysb = ysbs[key]
                yp = ypsum(f"yps{pair}_{t}_{o}", nslots)
                for i in range(ERCH):
                    nc.tensor.matmul(
                        yp, hmT[i][:, 128 * t:128 * t + 128],
                        bwo[:, o, i, :],
                        start=(i == 0), stop=(i == ERCH - 1))
                o0 = 512 * o
                nc.scalar.copy(ysb[:, o0:o0 + 256], yp[:, 0:256])
                nc.vector.tensor_copy(ysb[:, o0 + 256:o0 + 512],
                                      yp[:, 256:512])

            def y_out(pair, t, eng, split=False):
                ysb = ysbs[(pair, t)]
                if split:
                    for o in range(NOCH):
                        e2 = (nc.sync, nc.scalar, nc.gpsimd, nc.scalar)[o]
                        o0 = 512 * o
                        e2.dma_start(out=yo.ap()[pair, t][:, o0:o0 + 512],
                                     in_=ysb[:, o0:o0 + 512])
                else:
                    eng.dma_start(out=yo.ap()[pair, t], in_=ysb)

            # ================= schedule =================
            # PE order: dummies | g0+h0 k-major (dense from the start) |
            # tr0 | g1 | masktr0 | h1 (into y0/y1/lg/tr banks) | y0 chains
            # with pair-1 epilogue PE ops slotted in | y1 chains.
            lg0, hps0 = gating_and_h0()
            issue_b()
            lg_sb0 = lg_copy(lg0)
            eptr0 = tr_logits(0, lg_sb0)
            lg1 = gating1()
            mval0 = softmax_mask(0, eptr0)
            lg_sb1 = lg_copy(lg1)
            mvT0 = mask_transpose(0, eptr0, mval0)
            mexp0 = mask_expand(0, mvT0, nc.sync)
            hps1 = h1_block()
            hm0 = hm_mask(0, hps0, mexp0)

            youtq = (nc.sync, nc.scalar, nc.gpsimd, nc.sync)
            epi1 = {}
            for t in range(NT):
                for o in range(NOCH):
                    y_chain(0, t, o, hm0, 3)
                    # slot pair-1 epilogue PE ops between early chains so
                    # they don't block the chain stream
                    if t == 0 and o == 1:
                        epi1["eptr"] = tr_logits(1, lg_sb1)
                    if t == 0 and o == 3:
                        epi1["mval"] = softmax_mask(1, epi1["eptr"])
                    if t == 1 and o == 2:
                        mvT1 = mask_transpose(1, epi1["eptr"], epi1["mval"])
                        epi1["mexp"] = mask_expand(1, mvT1, nc.gpsimd)
                y_out(0, t, youtq[t])

            hm1 = hm_mask(1, hps1, epi1["mexp"])
            for t in range(NT):
                for o in range(NOCH):
                    y_chain(1, t, o, hm1, 4)
                if t < NT - 1:
                    y_out(1, t, (nc.scalar, nc.gpsimd, nc.sync)[t])
                else:
                    y_out(1, t, None, split=True)

    nc.compile()
    _CACHE["nc"] = nc
    return nc


def _prep_inputs(x, A, Bw, Wg):
    xf = np.asarray(x, dtype=np.float32).reshape(N, D)
    xT = np.ascontiguousarray(xf.T).astype(np.float16)           # [D, N]
    A_t = np.asarray(A, dtype=np.float32).reshape(ER, D).T       # [D, ER]
    af = np.ascontiguousarray(
        A_t.reshape(KD, 128, ER).transpose(1, 0, 2)).astype(np.float16)
    Bwt = (np.asarray(Bw, dtype=np.float32).transpose(0, 2, 1).reshape(ER, D)
           * SCALING)
    bw = np.ascontiguousarray(
        Bwt.reshape(ERCH, 128, NOCH, 512).transpose(1, 2, 0, 3)
    ).astype(np.float16)
    WgT = np.asarray(Wg, dtype=np.float32).T                     # [D, E]
    wg = np.ascontiguousarray(
        WgT.reshape(KD, 128, E).transpose(1, 0, 2)).astype(np.float16)
    in_maps = []
    for c in range(NCORES):
        xc = np.ascontiguousarray(
            xT[:, c * NTOK:(c + 1) * NTOK].reshape(KD, 128, NTOK)
            .transpose(1, 0, 2))
        in_maps.append({"xt": xc, "af": af, "bw": bw, "wg": wg})
    return in_maps


def _run(x, A, Bw, Wg, trace=False):
    nc = _build()
    in_maps = _prep_inputs(x, A, Bw, Wg)
    res = bass_utils.run_bass_kernel_spmd(
        nc, in_maps, core_ids=list(range(NCORES)), trace=trace)
    y = np.concatenate(
        [np.asarray(res.results[c]["yo"], dtype=np.float32).reshape(NTOK, D)
         for c in range(NCORES)], axis=0)
    return y.reshape(B, S, D), res


def kernel(x, A, Bw, Wg):
    y, _ = _run(x, A, Bw, Wg, trace=False)
    return y


# revision 19
# speedup vs baseline: 1.0516x; 1.0516x over previous
"""MoE low-rank adapters (top-1 routing) Trainium2 kernel.

Math (reference):
  xf = x.reshape(N, D)                 N=8192, D=2048, E=8, R=64
  logits = xf @ Wg.T                   [N, E]
  prob = softmax(logits); gate = argmax(prob); prob_sel = max(prob)
  h = xf @ A[e].T for all e            [N, E*R]
  y = (h * onehot(gate)) @ Bwt         [N, D]
  y *= SCALING * prob_sel

Distribution: data-parallel over tokens, 8 cores x 1024 tokens.

Design (v9, fp16 end-to-end):
- Everything lives in fp16: x (host-cast; gating argmax verified safe in
  fp16 with ~40x margin on the worst top-2 logit gap), A, Bw, Wg, y, and
  the mask staging. Halves x DMA vs f32 and removes the on-chip
  f32->bf16 cast stage entirely: h matmuls read the same x tiles as
  gating. Total DMA ~12.3MB/core vs ~19MB in the f32r/bf16 version.
- Pair-0 front is k-major: per k-chunk, one gating matmul plus the four
  h-bank matmuls consume the x/A granule that just streamed in. PE goes
  continuously busy right after a short warmup, so the HAM clock ramps
  to 2.4GHz early and stays there (gaps are what hold it at 1.2GHz).
- PE order: warmup, {g0,h0}x16, [epilogue0], g1, y0[0..14], h1,
  y0[15] (deferred chain fills the hm1 wait), y1. Softmax/mask epilogue
  ops slot between chains; the pair-1 mask DRAM broadcast rides gpsimd
  while y0 runs.
- PSUM: 4 h banks + lg + tr + 2 y banks = 8. y chains rotate over three
  slots (y0, y1, + lg during pair-0 emit / + h0 bank during pair-1
  emit) so bank-release copies stay off the next chain's critical path.
- DMA: 256KB-512KB pieces round-robined over the three queues
  (sync/scalar HWDGE, gpsimd SWDGE) in priority order: wg, {x0,A}
  interleaved k-granules, x1, mask0 staging, B per-o pieces, y-out per
  tok-chunk. Final tok-chunk drains per-o across all three queues.
"""

import sys

for _p in ("/opt/trn_rl_repo",):
    if _p not in sys.path:
        sys.path.insert(0, _p)

import numpy as np

import concourse.bass as bass
import concourse.bacc as bacc
import concourse.mybir as mybir
import concourse.tile as tile
from concourse import bass_utils
from concourse.masks import make_identity

f32 = mybir.dt.float32
f16 = mybir.dt.float16
bf16 = mybir.dt.bfloat16

B, S, D, R, E = 4, 2048, 2048, 64, 8
N = B * S                    # 8192 tokens
NCORES = 8
NTOK = N // NCORES           # 1024 tokens per core
SCALING = 64.0 / 16.0
ER = E * R                   # 512
KD = D // 128                # 16 d-chunks
PBLK = 512                   # tokens per pair-block
NPAIR = NTOK // PBLK         # 2
ERCH = ER // 128             # er chunks (4)
NT = PBLK // 128             # tok-chunks per pair (4)
NOCH = D // 512              # output chunks of 512 (4)
KG = 2                       # k-chunks per DMA granule

_CACHE = {}


def _build():
    if "nc" in _CACHE:
        return _CACHE["nc"]
    nc = bacc.Bacc("TRN2", target_bir_lowering=False, debug=False,
                   num_devices=NCORES)
    xt = nc.dram_tensor("xt", [128, KD, NTOK], f16, kind="ExternalInput")
    af = nc.dram_tensor("af", [128, KD, ER], f16, kind="ExternalInput")
    bw = nc.dram_tensor("bw", [128, NOCH, ERCH, 512], f16, kind="ExternalInput")
    wg = nc.dram_tensor("wg", [128, KD, E], f16, kind="ExternalInput")
    yo = nc.dram_tensor("yo", [NPAIR, NT, 128, D], f16, kind="ExternalOutput")
    mstage = nc.dram_tensor("mstage", [NPAIR, NT * E, 128], f16,
                            kind="Internal")

    with tile.TileContext(nc) as tc:
        import contextlib
        ctx = contextlib.ExitStack()
        with ctx:
            singles = ctx.enter_context(tc.tile_pool(name="singles", bufs=1))
            hpool = ctx.enter_context(tc.tile_pool(name="hpool", bufs=2))
            mpool = ctx.enter_context(tc.tile_pool(name="mpool", bufs=2))
            spool = ctx.enter_context(tc.tile_pool(name="spool", bufs=2))
            ypool = ctx.enter_context(tc.tile_pool(name="ypool", bufs=4))
            ps_h = ctx.enter_context(tc.tile_pool(name="ps_h", bufs=1, space="PSUM"))
            ps_lg = ctx.enter_context(tc.tile_pool(name="ps_lg", bufs=1, space="PSUM"))
            ps_tr = ctx.enter_context(tc.tile_pool(name="ps_tr", bufs=1, space="PSUM"))
            ps_y = ctx.enter_context(tc.tile_pool(name="ps_y", bufs=1, space="PSUM"))

            qs = (nc.sync, nc.scalar, nc.gpsimd)

            # ---- identities + gate weights ----
            ident = singles.tile([128, 128], f32)
            make_identity(nc, ident)
            identb = singles.tile([128, 128], bf16)
            make_identity(nc, identb)
            wg_sb = singles.tile([128, KD, E], f16)
            nc.sync.dma_start(out=wg_sb, in_=wg.ap())

            # ---- big fp16 SBUF tiles; DMAs land in slices ----
            xs = singles.tile([128, KD, NTOK], f16)
            ab = singles.tile([128, KD, ER], f16)
            bwo = singles.tile([128, NOCH, ERCH, 512], f16)

            # phase 0: full-token-width x granules (2KB runs), then A
            qi = [0]

            def q():
                e = qs[qi[0] % 3]
                qi[0] += 1
                return e

            # x and A granules interleaved: the k-major g+h front consumes
            # x_k and A_k together, so they must arrive together
            for g in range(KD // KG):
                k0 = KG * g
                q().dma_start(out=xs[:, k0:k0 + KG, :],
                              in_=xt.ap()[:, k0:k0 + KG, :])
                q().dma_start(out=ab[:, k0:k0 + KG, :],
                              in_=af.ap()[:, k0:k0 + KG, :])

            def issue_b():
                # per-o pieces (4KB runs): y chain (t,o) reads bwo[:, o, i, :]
                for o in range(NOCH):
                    eng = (nc.scalar, nc.gpsimd, nc.scalar, nc.gpsimd)[o]
                    eng.dma_start(out=bwo[:, o], in_=bw.ap()[:, o])

            # ---- PE warm-up while the first granules stream in ----
            wup = ps_y.tile([128, 512], f32, tag="y0", name="warmup_ps")

            def dummy(n):
                # clock-keeper: PE stays busy through DMA-paced stretches so
                # the HAM clock ramps once and stays at 2.4GHz
                for _ in range(n):
                    nc.tensor.matmul(wup[:, 0:128], identb, identb,
                                     start=True, stop=True)

            dummy(12)

            def gating_and_h0():
                # k-major: per k, gating + all four h banks consume the
                # granule that just landed -> PE continuously busy from the
                # start, HAM clock ramps once
                lg = ps_lg.tile([128, PBLK], f32, tag="lg", name="lg0")
                hps = [ps_h.tile([128, PBLK], f32, tag=f"h{i}",
                                 name=f"h0_{i}") for i in range(ERCH)]
                for k in range(KD):
                    nc.tensor.matmul(
                        lg[0:E, :], wg_sb[:, k, :], xs[:, k, 0:PBLK],
                        start=(k == 0), stop=(k == KD - 1))
                    for i in range(ERCH):
                        nc.tensor.matmul(
                            hps[i], ab[:, k, 128 * i:128 * i + 128],
                            xs[:, k, 0:PBLK],
                            start=(k == 0), stop=(k == KD - 1))
                return lg, hps

            def gating1():
                lg = ps_lg.tile([128, PBLK], f32, tag="lg", name="lg1")
                for k in range(KD):
                    nc.tensor.matmul(
                        lg[0:E, :], wg_sb[:, k, :], xs[:, k, PBLK:NTOK],
                        start=(k == 0), stop=(k == KD - 1))
                return lg

            def h1_block():
                # pair-1 h into the four non-h PSUM banks (y0/y1/lg/tr) so
                # it runs back-to-back after g1 with no mask dependency
                hps = [ps_y.tile([128, PBLK], f32, tag="y0", name="h1_0"),
                       ps_y.tile([128, PBLK], f32, tag="y1", name="h1_1"),
                       ps_lg.tile([128, PBLK], f32, tag="lg", name="h1_2"),
                       ps_tr.tile([128, PBLK], f32, tag="tr", name="h1_3")]
                for k in range(KD):
                    for i in range(ERCH):
                        nc.tensor.matmul(
                            hps[i], ab[:, k, 128 * i:128 * i + 128],
                            xs[:, k, PBLK:NTOK],
                            start=(k == 0), stop=(k == KD - 1))
                return hps

            def lg_copy(lg):
                lg_sb = spool.tile([E, PBLK], f32, tag="lg_sb")
                nc.scalar.copy(lg_sb, lg[0:E, :])
                return lg_sb

            def tr_logits(pair, lg_sb):
                # pair 1 can't use the tr bank (h1 holds it until hm1, which
                # transitively needs this transpose) -> use h3, free by then
                pool, tag = (ps_tr, "tr") if pair == 0 else (ps_h, "h3")
                eptr = pool.tile([128, 512], f32, tag=tag, name=f"tr{pair}")
                for t in range(NT):
                    nc.tensor.transpose(
                        eptr[:, 8 * t:8 * t + 8],
                        lg_sb[:, 128 * t:128 * t + 128], ident[0:E, 0:E])
                return eptr

            def softmax_mask(pair, eptr):
                mxs = []
                for t in range(NT):
                    mx = spool.tile([128, 1], f32, tag=f"mx{t}")
                    nc.vector.reduce_max(out=mx, in_=eptr[:, 8 * t:8 * t + 8],
                                         axis=mybir.AxisListType.X)
                    mxs.append(mx)
                negs = []
                for t in range(NT):
                    negmx = spool.tile([128, 1], f32, tag=f"negmx{t}")
                    nc.vector.tensor_scalar_mul(negmx, mxs[t], -1.0)
                    negs.append(negmx)
                ses = []
                for t in range(NT):
                    es = spool.tile([128, 8], f32, tag=f"es{t}")
                    se = spool.tile([128, 1], f32, tag=f"se{t}")
                    nc.scalar.activation(out=es, in_=eptr[:, 8 * t:8 * t + 8],
                                         func=mybir.ActivationFunctionType.Exp,
                                         bias=negs[t], scale=1.0, accum_out=se)
                    ses.append(se)
                rcps = []
                for t in range(NT):
                    rcp = spool.tile([128, 1], f32, tag=f"rcp{t}")
                    nc.vector.reciprocal(rcp, ses[t])
                    rcps.append(rcp)
                mval4 = spool.tile([128, NT, 8], f32, tag="mval4")
                for t in range(NT):
                    nc.vector.tensor_scalar(
                        out=mval4[:, t, :], in0=eptr[:, 8 * t:8 * t + 8],
                        scalar1=mxs[t], scalar2=rcps[t],
                        op0=mybir.AluOpType.is_equal,
                        op1=mybir.AluOpType.mult)
                return mval4

            def mask_transpose(pair, eptr, mval4):
                # one fused transpose [128 tok, (t e)=32] -> [32, 128]
                nc.tensor.transpose(eptr[0:32, 128:256],
                                    mval4.rearrange("p t e -> p (t e)"), ident)
                mvT4 = mpool.tile([32, 128], f16, tag="mvT4")
                nc.scalar.copy(mvT4, eptr[0:32, 128:256])
                return mvT4

            def mask_expand(pair, mvT4, eng):
                # stage to DRAM, broadcast over the 64 ranks of each expert;
                # the 8 broadcast pieces spread across all three queues so
                # the expansion takes ~2us instead of ~5us serial
                eng.dma_start(out=mstage.ap()[pair], in_=mvT4)
                mexp = []
                for i in range(ERCH):
                    me = mpool.tile([128, PBLK], f16, tag=f"me{i}")
                    mexp.append(me)
                    for half_e in range(2):
                        srcap = bass.AP(
                            tensor=mstage,
                            offset=(pair * (NT * E * 128)
                                    + (2 * i + half_e) * 128),
                            ap=[[0, 64], [E * 128, NT], [1, 128]],
                        )
                        qs[(2 * i + half_e) % 3].dma_start(
                            out=me[64 * half_e:64 * half_e + 64, :]
                            .rearrange("p (t n) -> p t n", t=NT),
                            in_=srcap)
                return mexp

            def hm_mask(pair, hps, mexp):
                hm = []
                for i in range(ERCH):
                    t_ = hpool.tile([128, PBLK], f16, tag=f"hm{i}",
                                    name=f"hm{pair}_{i}")
                    nc.vector.tensor_mul(t_, hps[i], mexp[i])
                    hm.append(t_)
                return hm

            yrot = [0]

            def ypsum(name, nslots):
                # y chains rotate over h banks (free after the hm muls);
                # h3 is reserved for pair-1's epilogue during the y0 phase
                tag = f"h{yrot[0] % nslots}"
                yrot[0] += 1
                return ps_h.tile([128, 512], f32, tag=tag, name=name)

            ysbs = {}

            def y_chain(pair, t, o, hmT, nslots):
                key = (pair, t)
                if key not in ysbs:
                    ysbs[key] = ypool.tile([128, D], f16, tag="ysb",
                                           name=f"ysb{pair}_{t}")
                ysb = ysbs[key]
                yp = ypsum(f"yps{pair}_{t}_{o}", nslots)
                for i in range(ERCH):
                    nc.tensor.matmul(
                        yp, hmT[i][:, 128 * t:128 * t + 128],
                        bwo[:, o, i, :],
                        start=(i == 0), stop=(i == ERCH - 1))
                o0 = 512 * o
                nc.scalar.copy(ysb[:, o0:o0 + 256], yp[:, 0:256])
                nc.vector.tensor_copy(ysb[:, o0 + 256:o0 + 512],
                                      yp[:, 256:512])

            def y_out(pair, t, eng, split=False):
                ysb = ysbs[(pair, t)]
                if split:
                    for o in range(NOCH):
                        e2 = (nc.sync, nc.scalar, nc.gpsimd, nc.scalar)[o]
                        o0 = 512 * o
                        e2.dma_start(out=yo.ap()[pair, t][:, o0:o0 + 512],
                                     in_=ysb[:, o0:o0 + 512])
                else:
                    eng.dma_start(out=yo.ap()[pair, t], in_=ysb)

            # ================= schedule =================
            # PE order: dummies | g0+h0 k-major (dense from the start) |
            # tr0 | g1 | masktr0 | h1 (into y0/y1/lg/tr banks) | y0 chains
            # with pair-1 epilogue PE ops slotted in | y1 chains.
            lg0, hps0 = gating_and_h0()
            issue_b()
            lg_sb0 = lg_copy(lg0)
            eptr0 = tr_logits(0, lg_sb0)
            lg1 = gating1()
            mval0 = softmax_mask(0, eptr0)
            lg_sb1 = lg_copy(lg1)
            mvT0 = mask_transpose(0, eptr0, mval0)
            mexp0 = mask_expand(0, mvT0, nc.sync)
            hps1 = h1_block()
            hm0 = hm_mask(0, hps0, mexp0)

            youtq = (nc.sync, nc.scalar, nc.gpsimd, nc.sync)
            epi1 = {}
            for t in range(NT):
                for o in range(NOCH):
                    y_chain(0, t, o, hm0, 3)
                    # slot pair-1 epilogue PE ops between early chains so
                    # they don't block the chain stream
                    if t == 0 and o == 1:
                        epi1["eptr"] = tr_logits(1, lg_sb1)
                    if t == 0 and o == 3:
                        epi1["mval"] = softmax_mask(1, epi1["eptr"])
                    if t == 1 and o == 2:
                        mvT1 = mask_transpose(1, epi1["eptr"], epi1["mval"])
                        epi1["mexp"] = mask_expand(1, mvT1, nc.gpsimd)
                y_out(0, t, youtq[t])

            hm1 = hm_mask(1, hps1, epi1["mexp"])
            for t in range(NT):
                for o in range(NOCH):
                    y_chain(1, t, o, hm1, 4)
                if t < NT - 1:
                    y_out(1, t, (nc.scalar, nc.gpsimd, nc.sync)[t])
                else:
                    y_out(1, t, None, split=True)

    nc.compile()
    _CACHE["nc"] = nc
    return nc


def _prep_inputs(x, A, Bw, Wg):
    xf = np.asarray(x, dtype=np.float32).reshape(N, D)
    xT = np.ascontiguousarray(xf.T).astype(np.float16)           # [D, N]
    A_t = np.asarray(A, dtype=np.float32).reshape(ER, D).T       # [D, ER]
    af = np.ascontiguousarray(
        A_t.reshape(KD, 128, ER).transpose(1, 0, 2)).astype(np.float16)
    Bwt = (np.asarray(Bw, dtype=np.float32).transpose(0, 2, 1).reshape(ER, D)
           * SCALING)
    bw = np.ascontiguousarray(
        Bwt.reshape(ERCH, 128, NOCH, 512).transpose(1, 2, 0, 3)
    ).astype(np.float16)
    WgT = np.asarray(Wg, dtype=np.float32).T                     # [D, E]
    wg = np.ascontiguousarray(
        WgT.reshape(KD, 128, E).transpose(1, 0, 2)).astype(np.float16)
    in_maps = []
    for c in range(NCORES):
        xc = np.ascontiguousarray(
            xT[:, c * NTOK:(c + 1) * NTOK].reshape(KD, 128, NTOK)
            .transpose(1, 0, 2))
        in_maps.append({"xt": xc, "af": af, "bw": bw, "wg": wg})
    return in_maps


def _run(x, A, Bw, Wg, trace=False):
    nc = _build()
    in_maps = _prep_inputs(x, A, Bw, Wg)
    res = bass_utils.run_bass_kernel_spmd(
        nc, in_maps, core_ids=list(range(NCORES)), trace=trace)
    y = np.concatenate(
        [np.asarray(res.results[c]["yo"], dtype=np.float32).reshape(NTOK, D)
         for c in range(NCORES)], axis=0)
    return y.reshape(B, S, D), res


def kernel(x, A, Bw, Wg):
    y, _ = _run(x, A, Bw, Wg, trace=False)
    return y


# revision 21
# speedup vs baseline: 1.1101x; 1.0556x over previous
"""MoE low-rank adapters (top-1 routing) Trainium2 kernel.

Math (reference):
  xf = x.reshape(N, D)                 N=8192, D=2048, E=8, R=64
  logits = xf @ Wg.T                   [N, E]
  prob = softmax(logits); gate = argmax(prob); prob_sel = max(prob)
  h = xf @ A[e].T for all e            [N, E*R]
  y = (h * onehot(gate)) @ Bwt         [N, D]
  y *= SCALING * prob_sel

Distribution: data-parallel over tokens, 8 cores x 1024 tokens.

Design (v9, fp16 end-to-end):
- Everything lives in fp16: x (host-cast; gating argmax verified safe in
  fp16 with ~40x margin on the worst top-2 logit gap), A, Bw, Wg, y, and
  the mask staging. Halves x DMA vs f32 and removes the on-chip
  f32->bf16 cast stage entirely: h matmuls read the same x tiles as
  gating. Total DMA ~12.3MB/core vs ~19MB in the f32r/bf16 version.
- Pair-0 front is k-major: per k-chunk, one gating matmul plus the four
  h-bank matmuls consume the x/A granule that just streamed in. PE goes
  continuously busy right after a short warmup, so the HAM clock ramps
  to 2.4GHz early and stays there (gaps are what hold it at 1.2GHz).
- PE order: warmup, {g0,h0}x16, [epilogue0], g1, y0[0..14], h1,
  y0[15] (deferred chain fills the hm1 wait), y1. Softmax/mask epilogue
  ops slot between chains; the pair-1 mask DRAM broadcast rides gpsimd
  while y0 runs.
- PSUM: 4 h banks + lg + tr + 2 y banks = 8. y chains rotate over three
  slots (y0, y1, + lg during pair-0 emit / + h0 bank during pair-1
  emit) so bank-release copies stay off the next chain's critical path.
- DMA: 256KB-512KB pieces round-robined over the three queues
  (sync/scalar HWDGE, gpsimd SWDGE) in priority order: wg, {x0,A}
  interleaved k-granules, x1, mask0 staging, B per-o pieces, y-out per
  tok-chunk. Final tok-chunk drains per-o across all three queues.
"""

import sys

for _p in ("/opt/trn_rl_repo",):
    if _p not in sys.path:
        sys.path.insert(0, _p)

import numpy as np

import concourse.bass as bass
import concourse.bacc as bacc
import concourse.mybir as mybir
import concourse.tile as tile
from concourse import bass_utils
from concourse.masks import make_identity

f32 = mybir.dt.float32
f16 = mybir.dt.float16
bf16 = mybir.dt.bfloat16

B, S, D, R, E = 4, 2048, 2048, 64, 8
N = B * S                    # 8192 tokens
NCORES = 8
NTOK = N // NCORES           # 1024 tokens per core
SCALING = 64.0 / 16.0
ER = E * R                   # 512
KD = D // 128                # 16 d-chunks
PBLK = 512                   # tokens per pair-block
NPAIR = NTOK // PBLK         # 2
ERCH = ER // 128             # er chunks (4)
NT = PBLK // 128             # tok-chunks per pair (4)
NOCH = D // 512              # output chunks of 512 (4)
KG = 2                       # k-chunks per DMA granule

_CACHE = {}


def _build():
    if "nc" in _CACHE:
        return _CACHE["nc"]
    nc = bacc.Bacc("TRN2", target_bir_lowering=False, debug=False,
                   num_devices=NCORES)
    xt = nc.dram_tensor("xt", [128, KD, NTOK], f16, kind="ExternalInput")
    af = nc.dram_tensor("af", [128, KD, ER], f16, kind="ExternalInput")
    bw = nc.dram_tensor("bw", [128, NOCH, ERCH, 512], f16, kind="ExternalInput")
    wg = nc.dram_tensor("wg", [128, KD, E], f16, kind="ExternalInput")
    yo = nc.dram_tensor("yo", [NPAIR, NT, 128, D], f16, kind="ExternalOutput")
    mstage = nc.dram_tensor("mstage", [NPAIR, NT * E, 128], f16,
                            kind="Internal")

    with tile.TileContext(nc) as tc:
        import contextlib
        ctx = contextlib.ExitStack()
        with ctx:
            singles = ctx.enter_context(tc.tile_pool(name="singles", bufs=1))
            hpool = ctx.enter_context(tc.tile_pool(name="hpool", bufs=2))
            mpool = ctx.enter_context(tc.tile_pool(name="mpool", bufs=2))
            spool = ctx.enter_context(tc.tile_pool(name="spool", bufs=2))
            ypool = ctx.enter_context(tc.tile_pool(name="ypool", bufs=4))
            ps_h = ctx.enter_context(tc.tile_pool(name="ps_h", bufs=1, space="PSUM"))
            ps_lg = ctx.enter_context(tc.tile_pool(name="ps_lg", bufs=1, space="PSUM"))
            ps_tr = ctx.enter_context(tc.tile_pool(name="ps_tr", bufs=1, space="PSUM"))
            ps_y = ctx.enter_context(tc.tile_pool(name="ps_y", bufs=1, space="PSUM"))

            qs = (nc.sync, nc.scalar, nc.gpsimd)

            # ---- identities + gate weights ----
            ident = singles.tile([128, 128], f32)
            make_identity(nc, ident)
            identb = singles.tile([128, 128], bf16)
            make_identity(nc, identb)
            wg_sb = singles.tile([128, KD, E], f16)
            nc.sync.dma_start(out=wg_sb, in_=wg.ap())

            # ---- big fp16 SBUF tiles; DMAs land in slices ----
            xs = singles.tile([128, KD, NTOK], f16)
            ab = singles.tile([128, KD, ER], f16)
            bwo = singles.tile([128, NOCH, ERCH, 512], f16)

            # phase 0: full-token-width x granules (2KB runs), then A
            qi = [0]

            def q():
                e = qs[qi[0] % 3]
                qi[0] += 1
                return e

            # x and A granules interleaved: the k-major g+h front consumes
            # x_k and A_k together, so they must arrive together
            for g in range(KD // KG):
                k0 = KG * g
                q().dma_start(out=xs[:, k0:k0 + KG, :],
                              in_=xt.ap()[:, k0:k0 + KG, :])
                q().dma_start(out=ab[:, k0:k0 + KG, :],
                              in_=af.ap()[:, k0:k0 + KG, :])

            def issue_b():
                # per-o pieces (4KB runs): y chain (t,o) reads bwo[:, o, i, :]
                for o in range(NOCH):
                    eng = (nc.scalar, nc.gpsimd, nc.scalar, nc.gpsimd)[o]
                    eng.dma_start(out=bwo[:, o], in_=bw.ap()[:, o])

            # ---- PE warm-up while the first granules stream in ----
            wup = ps_y.tile([128, 512], f32, tag="y0", name="warmup_ps")

            def dummy(n):
                # clock-keeper: PE stays busy through DMA-paced stretches so
                # the HAM clock ramps once and stays at 2.4GHz
                for _ in range(n):
                    nc.tensor.matmul(wup[:, 0:128], identb, identb,
                                     start=True, stop=True)

            dummy(28)

            def gating_and_h0():
                # k-major: per k, gating + all four h banks consume the
                # granule that just landed -> PE continuously busy from the
                # start, HAM clock ramps once
                lg = ps_lg.tile([128, PBLK], f32, tag="lg", name="lg0")
                hps = [ps_h.tile([128, PBLK], f32, tag=f"h{i}",
                                 name=f"h0_{i}") for i in range(ERCH)]
                for k in range(KD):
                    nc.tensor.matmul(
                        lg[0:E, :], wg_sb[:, k, :], xs[:, k, 0:PBLK],
                        start=(k == 0), stop=(k == KD - 1))
                    for i in range(ERCH):
                        nc.tensor.matmul(
                            hps[i], ab[:, k, 128 * i:128 * i + 128],
                            xs[:, k, 0:PBLK],
                            start=(k == 0), stop=(k == KD - 1))
                    if k < 10:
                        # DMA-paced stretch: keep PE busy so the clock holds
                        dummy(2)
                return lg, hps

            def gating1():
                lg = ps_lg.tile([128, PBLK], f32, tag="lg", name="lg1")
                for k in range(KD):
                    nc.tensor.matmul(
                        lg[0:E, :], wg_sb[:, k, :], xs[:, k, PBLK:NTOK],
                        start=(k == 0), stop=(k == KD - 1))
                return lg

            def h1_block():
                # pair-1 h into the four non-h PSUM banks (y0/y1/lg/tr) so
                # it runs back-to-back after g1 with no mask dependency
                hps = [ps_y.tile([128, PBLK], f32, tag="y0", name="h1_0"),
                       ps_y.tile([128, PBLK], f32, tag="y1", name="h1_1"),
                       ps_lg.tile([128, PBLK], f32, tag="lg", name="h1_2"),
                       ps_tr.tile([128, PBLK], f32, tag="tr", name="h1_3")]
                for k in range(KD):
                    for i in range(ERCH):
                        nc.tensor.matmul(
                            hps[i], ab[:, k, 128 * i:128 * i + 128],
                            xs[:, k, PBLK:NTOK],
                            start=(k == 0), stop=(k == KD - 1))
                return hps

            def lg_copy(lg):
                lg_sb = spool.tile([E, PBLK], f32, tag="lg_sb")
                nc.scalar.copy(lg_sb, lg[0:E, :])
                return lg_sb

            def tr_logits(pair, lg_sb):
                # pair 1 can't use the tr bank (h1 holds it until hm1, which
                # transitively needs this transpose) -> use h3, free by then
                pool, tag = (ps_tr, "tr") if pair == 0 else (ps_h, "h3")
                eptr = pool.tile([128, 512], f32, tag=tag, name=f"tr{pair}")
                for t in range(NT):
                    nc.tensor.transpose(
                        eptr[:, 8 * t:8 * t + 8],
                        lg_sb[:, 128 * t:128 * t + 128], ident[0:E, 0:E])
                return eptr

            def softmax_mask(pair, eptr):
                mxs = []
                for t in range(NT):
                    mx = spool.tile([128, 1], f32, tag=f"mx{t}")
                    nc.vector.reduce_max(out=mx, in_=eptr[:, 8 * t:8 * t + 8],
                                         axis=mybir.AxisListType.X)
                    mxs.append(mx)
                negs = []
                for t in range(NT):
                    negmx = spool.tile([128, 1], f32, tag=f"negmx{t}")
                    nc.vector.tensor_scalar_mul(negmx, mxs[t], -1.0)
                    negs.append(negmx)
                ses = []
                for t in range(NT):
                    es = spool.tile([128, 8], f32, tag=f"es{t}")
                    se = spool.tile([128, 1], f32, tag=f"se{t}")
                    nc.scalar.activation(out=es, in_=eptr[:, 8 * t:8 * t + 8],
                                         func=mybir.ActivationFunctionType.Exp,
                                         bias=negs[t], scale=1.0, accum_out=se)
                    ses.append(se)
                rcps = []
                for t in range(NT):
                    rcp = spool.tile([128, 1], f32, tag=f"rcp{t}")
                    nc.vector.reciprocal(rcp, ses[t])
                    rcps.append(rcp)
                mval4 = spool.tile([128, NT, 8], f32, tag="mval4")
                for t in range(NT):
                    nc.vector.tensor_scalar(
                        out=mval4[:, t, :], in0=eptr[:, 8 * t:8 * t + 8],
                        scalar1=mxs[t], scalar2=rcps[t],
                        op0=mybir.AluOpType.is_equal,
                        op1=mybir.AluOpType.mult)
                return mval4

            def mask_transpose(pair, eptr, mval4):
                # one fused transpose [128 tok, (t e)=32] -> [32, 128]
                nc.tensor.transpose(eptr[0:32, 128:256],
                                    mval4.rearrange("p t e -> p (t e)"), ident)
                mvT4 = mpool.tile([32, 128], f16, tag="mvT4")
                nc.scalar.copy(mvT4, eptr[0:32, 128:256])
                return mvT4

            def mask_expand(pair, mvT4, eng):
                # stage to DRAM, broadcast over the 64 ranks of each expert;
                # the 8 broadcast pieces spread across all three queues so
                # the expansion takes ~2us instead of ~5us serial
                eng.dma_start(out=mstage.ap()[pair], in_=mvT4)
                mexp = []
                for i in range(ERCH):
                    me = mpool.tile([128, PBLK], f16, tag=f"me{i}")
                    mexp.append(me)
                    for half_e in range(2):
                        srcap = bass.AP(
                            tensor=mstage,
                            offset=(pair * (NT * E * 128)
                                    + (2 * i + half_e) * 128),
                            ap=[[0, 64], [E * 128, NT], [1, 128]],
                        )
                        qs[(2 * i + half_e) % 3].dma_start(
                            out=me[64 * half_e:64 * half_e + 64, :]
                            .rearrange("p (t n) -> p t n", t=NT),
                            in_=srcap)
                return mexp

            def hm_mask(pair, hps, mexp):
                hm = []
                for i in range(ERCH):
                    t_ = hpool.tile([128, PBLK], f16, tag=f"hm{i}",
                                    name=f"hm{pair}_{i}")
                    nc.vector.tensor_mul(t_, hps[i], mexp[i])
                    hm.append(t_)
                return hm

            yrot = [0]

            def ypsum(name, nslots):
                # y chains rotate over h banks (free after the hm muls);
                # h3 is reserved for pair-1's epilogue during the y0 phase
                tag = f"h{yrot[0] % nslots}"
                yrot[0] += 1
                return ps_h.tile([128, 512], f32, tag=tag, name=name)

            ysbs = {}

            def y_chain(pair, t, o, hmT, nslots):
                key = (pair, t)
                if key not in ysbs:
                    ysbs[key] = ypool.tile([128, D], f16, tag="ysb",
                                           name=f"ysb{pair}_{t}")
                ysb = ysbs[key]
                yp = ypsum(f"yps{pair}_{t}_{o}", nslots)
                for i in range(ERCH):
                    nc.tensor.matmul(
                        yp, hmT[i][:, 128 * t:128 * t + 128],
                        bwo[:, o, i, :],
                        start=(i == 0), stop=(i == ERCH - 1))
                o0 = 512 * o
                nc.scalar.copy(ysb[:, o0:o0 + 256], yp[:, 0:256])
                nc.vector.tensor_copy(ysb[:, o0 + 256:o0 + 512],
                                      yp[:, 256:512])

            def y_out(pair, t, eng, split=False):
                ysb = ysbs[(pair, t)]
                if split:
                    for o in range(NOCH):
                        e2 = (nc.sync, nc.scalar, nc.gpsimd, nc.scalar)[o]
                        o0 = 512 * o
                        e2.dma_start(out=yo.ap()[pair, t][:, o0:o0 + 512],
                                     in_=ysb[:, o0:o0 + 512])
                else:
                    eng.dma_start(out=yo.ap()[pair, t], in_=ysb)

            # ================= schedule =================
            # PE order: dummies | g0+h0 k-major (dense from the start) |
            # tr0 | g1 | masktr0 | h1 (into y0/y1/lg/tr banks) | y0 chains
            # with pair-1 epilogue PE ops slotted in | y1 chains.
            lg0, hps0 = gating_and_h0()
            issue_b()
            lg_sb0 = lg_copy(lg0)
            eptr0 = tr_logits(0, lg_sb0)
            lg1 = gating1()
            mval0 = softmax_mask(0, eptr0)
            lg_sb1 = lg_copy(lg1)
            mvT0 = mask_transpose(0, eptr0, mval0)
            mexp0 = mask_expand(0, mvT0, nc.sync)
            hps1 = h1_block()
            hm0 = hm_mask(0, hps0, mexp0)

            youtq = (nc.sync, nc.scalar, nc.gpsimd, nc.sync)
            epi1 = {}
            for t in range(NT):
                for o in range(NOCH):
                    y_chain(0, t, o, hm0, 3)
                    # slot pair-1 epilogue PE ops between early chains so
                    # they don't block the chain stream
                    if t == 0 and o == 1:
                        epi1["eptr"] = tr_logits(1, lg_sb1)
                    if t == 0 and o == 3:
                        epi1["mval"] = softmax_mask(1, epi1["eptr"])
                    if t == 1 and o == 2:
                        mvT1 = mask_transpose(1, epi1["eptr"], epi1["mval"])
                        epi1["mexp"] = mask_expand(1, mvT1, nc.gpsimd)
                y_out(0, t, youtq[t])

            hm1 = hm_mask(1, hps1, epi1["mexp"])
            for t in range(NT):
                for o in range(NOCH):
                    y_chain(1, t, o, hm1, 4)
                if t < NT - 1:
                    y_out(1, t, (nc.scalar, nc.gpsimd, nc.sync)[t])
                else:
                    y_out(1, t, None, split=True)

    nc.compile()
    _CACHE["nc"] = nc
    return nc


def _prep_inputs(x, A, Bw, Wg):
    xf = np.asarray(x, dtype=np.float32).reshape(N, D)
    xT = np.ascontiguousarray(xf.T).astype(np.float16)           # [D, N]
    A_t = np.asarray(A, dtype=np.float32).reshape(ER, D).T       # [D, ER]
    af = np.ascontiguousarray(
        A_t.reshape(KD, 128, ER).transpose(1, 0, 2)).astype(np.float16)
    Bwt = (np.asarray(Bw, dtype=np.float32).transpose(0, 2, 1).reshape(ER, D)
           * SCALING)
    bw = np.ascontiguousarray(
        Bwt.reshape(ERCH, 128, NOCH, 512).transpose(1, 2, 0, 3)
    ).astype(np.float16)
    WgT = np.asarray(Wg, dtype=np.float32).T                     # [D, E]
    wg = np.ascontiguousarray(
        WgT.reshape(KD, 128, E).transpose(1, 0, 2)).astype(np.float16)
    in_maps = []
    for c in range(NCORES):
        xc = np.ascontiguousarray(
            xT[:, c * NTOK:(c + 1) * NTOK].reshape(KD, 128, NTOK)
            .transpose(1, 0, 2))
        in_maps.append({"xt": xc, "af": af, "bw": bw, "wg": wg})
    return in_maps


def _run(x, A, Bw, Wg, trace=False):
    nc = _build()
    in_maps = _prep_inputs(x, A, Bw, Wg)
    res = bass_utils.run_bass_kernel_spmd(
        nc, in_maps, core_ids=list(range(NCORES)), trace=trace)
    y = np.concatenate(
        [np.asarray(res.results[c]["yo"], dtype=np.float32).reshape(NTOK, D)
         for c in range(NCORES)], axis=0)
    return y.reshape(B, S, D), res


def kernel(x, A, Bw, Wg):
    y, _ = _run(x, A, Bw, Wg, trace=False)
    return y


# revision 24
# speedup vs baseline: 1.1367x; 1.0240x over previous
"""MoE low-rank adapters (top-1 routing) Trainium2 kernel.

Math (reference):
  xf = x.reshape(N, D)                 N=8192, D=2048, E=8, R=64
  logits = xf @ Wg.T                   [N, E]
  prob = softmax(logits); gate = argmax(prob); prob_sel = max(prob)
  h = xf @ A[e].T for all e            [N, E*R]
  y = (h * onehot(gate)) @ Bwt         [N, D]
  y *= SCALING * prob_sel

Distribution: data-parallel over tokens, 8 cores x 1024 tokens.

Design (v9, fp16 end-to-end):
- Everything lives in fp16: x (host-cast; gating argmax verified safe in
  fp16 with ~40x margin on the worst top-2 logit gap), A, Bw, Wg, y, and
  the mask staging. Halves x DMA vs f32 and removes the on-chip
  f32->bf16 cast stage entirely: h matmuls read the same x tiles as
  gating. Total DMA ~12.3MB/core vs ~19MB in the f32r/bf16 version.
- Pair-0 front is k-major: per k-chunk, one gating matmul plus the four
  h-bank matmuls consume the x/A granule that just streamed in. PE goes
  continuously busy right after a short warmup, so the HAM clock ramps
  to 2.4GHz early and stays there (gaps are what hold it at 1.2GHz).
- PE order: warmup, {g0,h0}x16, [epilogue0], g1, y0[0..14], h1,
  y0[15] (deferred chain fills the hm1 wait), y1. Softmax/mask epilogue
  ops slot between chains; the pair-1 mask DRAM broadcast rides gpsimd
  while y0 runs.
- PSUM: 4 h banks + lg + tr + 2 y banks = 8. y chains rotate over three
  slots (y0, y1, + lg during pair-0 emit / + h0 bank during pair-1
  emit) so bank-release copies stay off the next chain's critical path.
- DMA: 256KB-512KB pieces round-robined over the three queues
  (sync/scalar HWDGE, gpsimd SWDGE) in priority order: wg, {x0,A}
  interleaved k-granules, x1, mask0 staging, B per-o pieces, y-out per
  tok-chunk. Final tok-chunk drains per-o across all three queues.
"""

import sys

for _p in ("/opt/trn_rl_repo",):
    if _p not in sys.path:
        sys.path.insert(0, _p)

import numpy as np

import concourse.bass as bass
import concourse.bacc as bacc
import concourse.mybir as mybir
import concourse.tile as tile
from concourse import bass_utils
from concourse.masks import make_identity

f32 = mybir.dt.float32
f16 = mybir.dt.float16
bf16 = mybir.dt.bfloat16

B, S, D, R, E = 4, 2048, 2048, 64, 8
N = B * S                    # 8192 tokens
NCORES = 8
NTOK = N // NCORES           # 1024 tokens per core
SCALING = 64.0 / 16.0
ER = E * R                   # 512
KD = D // 128                # 16 d-chunks
PBLK = 512                   # tokens per pair-block
NPAIR = NTOK // PBLK         # 2
ERCH = ER // 128             # er chunks (4)
NT = PBLK // 128             # tok-chunks per pair (4)
NOCH = D // 512              # output chunks of 512 (4)
KG = 2                       # k-chunks per DMA granule

_CACHE = {}


def _build():
    if "nc" in _CACHE:
        return _CACHE["nc"]
    nc = bacc.Bacc("TRN2", target_bir_lowering=False, debug=False,
                   num_devices=NCORES)
    xt = nc.dram_tensor("xt", [128, KD, NTOK], f16, kind="ExternalInput")
    af = nc.dram_tensor("af", [128, KD, ER], f16, kind="ExternalInput")
    bw = nc.dram_tensor("bw", [128, NOCH, ERCH, 512], f16, kind="ExternalInput")
    wg = nc.dram_tensor("wg", [128, KD, E], f16, kind="ExternalInput")
    yo = nc.dram_tensor("yo", [NPAIR, NT, 128, D], f16, kind="ExternalOutput")
    mstage = nc.dram_tensor("mstage", [NPAIR, NT * E, 128], f16,
                            kind="Internal")

    with tile.TileContext(nc) as tc:
        import contextlib
        ctx = contextlib.ExitStack()
        with ctx:
            singles = ctx.enter_context(tc.tile_pool(name="singles", bufs=1))
            hpool = ctx.enter_context(tc.tile_pool(name="hpool", bufs=2))
            mpool = ctx.enter_context(tc.tile_pool(name="mpool", bufs=2))
            spool = ctx.enter_context(tc.tile_pool(name="spool", bufs=2))
            ypool = ctx.enter_context(tc.tile_pool(name="ypool", bufs=4))
            ps_h = ctx.enter_context(tc.tile_pool(name="ps_h", bufs=1, space="PSUM"))
            ps_lg = ctx.enter_context(tc.tile_pool(name="ps_lg", bufs=1, space="PSUM"))
            ps_tr = ctx.enter_context(tc.tile_pool(name="ps_tr", bufs=1, space="PSUM"))
            ps_y = ctx.enter_context(tc.tile_pool(name="ps_y", bufs=1, space="PSUM"))

            qs = (nc.sync, nc.scalar, nc.gpsimd)

            # ---- identities + gate weights ----
            ident = singles.tile([128, 128], f32)
            make_identity(nc, ident)
            identb = singles.tile([128, 128], bf16)
            make_identity(nc, identb)
            wg_sb = singles.tile([128, KD, E], f16)
            nc.sync.dma_start(out=wg_sb, in_=wg.ap())

            # ---- big fp16 SBUF tiles; DMAs land in slices ----
            xs = singles.tile([128, KD, NTOK], f16)
            ab = singles.tile([128, KD, ER], f16)
            bwo = singles.tile([128, NOCH, ERCH, 512], f16)

            # phase 0: full-token-width x granules (2KB runs), then A
            qi = [0]

            def q():
                e = qs[qi[0] % 3]
                qi[0] += 1
                return e

            # x and A granules interleaved: the k-major g+h front consumes
            # x_k and A_k together, so they must arrive together
            for g in range(KD // KG):
                k0 = KG * g
                q().dma_start(out=xs[:, k0:k0 + KG, :],
                              in_=xt.ap()[:, k0:k0 + KG, :])
                q().dma_start(out=ab[:, k0:k0 + KG, :],
                              in_=af.ap()[:, k0:k0 + KG, :])

            def issue_b():
                # per-o pieces (4KB runs): y chain (t,o) reads bwo[:, o, i, :]
                for o in range(NOCH):
                    eng = (nc.scalar, nc.gpsimd, nc.scalar, nc.gpsimd)[o]
                    eng.dma_start(out=bwo[:, o], in_=bw.ap()[:, o])

            # ---- PE warm-up while the first granules stream in ----
            wup = ps_y.tile([128, 512], f32, tag="y0", name="warmup_ps")

            def dummy(n):
                # clock-keeper: PE stays busy through DMA-paced stretches so
                # the HAM clock ramps once and stays at 2.4GHz
                for _ in range(n):
                    nc.tensor.matmul(wup[:, 0:128], identb, identb,
                                     start=True, stop=True)

            dummy(40)

            def gating_and_h0():
                # k-major: per k, gating + all four h banks consume the
                # granule that just landed -> PE continuously busy from the
                # start, HAM clock ramps once
                lg = ps_lg.tile([128, PBLK], f32, tag="lg", name="lg0")
                hps = [ps_h.tile([128, PBLK], f32, tag=f"h{i}",
                                 name=f"h0_{i}") for i in range(ERCH)]
                for k in range(KD):
                    nc.tensor.matmul(
                        lg[0:E, :], wg_sb[:, k, :], xs[:, k, 0:PBLK],
                        start=(k == 0), stop=(k == KD - 1))
                    for i in range(ERCH):
                        nc.tensor.matmul(
                            hps[i], ab[:, k, 128 * i:128 * i + 128],
                            xs[:, k, 0:PBLK],
                            start=(k == 0), stop=(k == KD - 1))
                    if k < 8:
                        # DMA-paced stretch: keep PE busy so the clock holds
                        dummy(5)
                return lg, hps

            def gating1():
                lg = ps_lg.tile([128, PBLK], f32, tag="lg", name="lg1")
                for k in range(KD):
                    nc.tensor.matmul(
                        lg[0:E, :], wg_sb[:, k, :], xs[:, k, PBLK:NTOK],
                        start=(k == 0), stop=(k == KD - 1))
                return lg

            def h1_block():
                # pair-1 h into the four non-h PSUM banks (y0/y1/lg/tr) so
                # it runs back-to-back after g1 with no mask dependency
                hps = [ps_y.tile([128, PBLK], f32, tag="y0", name="h1_0"),
                       ps_y.tile([128, PBLK], f32, tag="y1", name="h1_1"),
                       ps_lg.tile([128, PBLK], f32, tag="lg", name="h1_2"),
                       ps_tr.tile([128, PBLK], f32, tag="tr", name="h1_3")]
                for k in range(KD):
                    for i in range(ERCH):
                        nc.tensor.matmul(
                            hps[i], ab[:, k, 128 * i:128 * i + 128],
                            xs[:, k, PBLK:NTOK],
                            start=(k == 0), stop=(k == KD - 1))
                return hps

            def lg_copy(lg):
                lg_sb = spool.tile([E, PBLK], f32, tag="lg_sb")
                nc.scalar.copy(lg_sb, lg[0:E, :])
                return lg_sb

            def tr_logits(pair, lg_sb):
                # pair 1 can't use the tr bank (h1 holds it until hm1, which
                # transitively needs this transpose) -> use h3, free by then
                pool, tag = (ps_tr, "tr") if pair == 0 else (ps_h, "h3")
                eptr = pool.tile([128, 512], f32, tag=tag, name=f"tr{pair}")
                for t in range(NT):
                    nc.tensor.transpose(
                        eptr[:, 8 * t:8 * t + 8],
                        lg_sb[:, 128 * t:128 * t + 128], ident[0:E, 0:E])
                return eptr

            def softmax_mask(pair, eptr):
                mxs = []
                for t in range(NT):
                    mx = spool.tile([128, 1], f32, tag=f"mx{t}")
                    nc.vector.reduce_max(out=mx, in_=eptr[:, 8 * t:8 * t + 8],
                                         axis=mybir.AxisListType.X)
                    mxs.append(mx)
                negs = []
                for t in range(NT):
                    negmx = spool.tile([128, 1], f32, tag=f"negmx{t}")
                    nc.vector.tensor_scalar_mul(negmx, mxs[t], -1.0)
                    negs.append(negmx)
                ses = []
                for t in range(NT):
                    es = spool.tile([128, 8], f32, tag=f"es{t}")
                    se = spool.tile([128, 1], f32, tag=f"se{t}")
                    nc.scalar.activation(out=es, in_=eptr[:, 8 * t:8 * t + 8],
                                         func=mybir.ActivationFunctionType.Exp,
                                         bias=negs[t], scale=1.0, accum_out=se)
                    ses.append(se)
                rcps = []
                for t in range(NT):
                    rcp = spool.tile([128, 1], f32, tag=f"rcp{t}")
                    nc.vector.reciprocal(rcp, ses[t])
                    rcps.append(rcp)
                mval4 = spool.tile([128, NT, 8], f32, tag="mval4")
                for t in range(NT):
                    nc.vector.tensor_scalar(
                        out=mval4[:, t, :], in0=eptr[:, 8 * t:8 * t + 8],
                        scalar1=mxs[t], scalar2=rcps[t],
                        op0=mybir.AluOpType.is_equal,
                        op1=mybir.AluOpType.mult)
                return mval4

            def mask_transpose(pair, eptr, mval4):
                # one fused transpose [128 tok, (t e)=32] -> [32, 128]
                nc.tensor.transpose(eptr[0:32, 128:256],
                                    mval4.rearrange("p t e -> p (t e)"), ident)
                mvT4 = mpool.tile([32, 128], f16, tag="mvT4")
                nc.scalar.copy(mvT4, eptr[0:32, 128:256])
                return mvT4

            def mask_expand(pair, mvT4, eng):
                # stage to DRAM, broadcast over the 64 ranks of each expert;
                # the 8 broadcast pieces spread across all three queues so
                # the expansion takes ~2us instead of ~5us serial
                eng.dma_start(out=mstage.ap()[pair], in_=mvT4)
                mexp = []
                for i in range(ERCH):
                    me = mpool.tile([128, PBLK], f16, tag=f"me{i}")
                    mexp.append(me)
                    for half_e in range(2):
                        srcap = bass.AP(
                            tensor=mstage,
                            offset=(pair * (NT * E * 128)
                                    + (2 * i + half_e) * 128),
                            ap=[[0, 64], [E * 128, NT], [1, 128]],
                        )
                        qs[(2 * i + half_e) % 3].dma_start(
                            out=me[64 * half_e:64 * half_e + 64, :]
                            .rearrange("p (t n) -> p t n", t=NT),
                            in_=srcap)
                return mexp

            def hm_mask(pair, hps, mexp):
                hm = []
                for i in range(ERCH):
                    t_ = hpool.tile([128, PBLK], f16, tag=f"hm{i}",
                                    name=f"hm{pair}_{i}")
                    nc.vector.tensor_mul(t_, hps[i], mexp[i])
                    hm.append(t_)
                return hm

            yrot = [0]

            def ypsum(name, nslots):
                # y chains rotate over h banks (free after the hm muls);
                # h3 is reserved for pair-1's epilogue during the y0 phase
                tag = f"h{yrot[0] % nslots}"
                yrot[0] += 1
                return ps_h.tile([128, 512], f32, tag=tag, name=name)

            ysbs = {}

            def y_chain(pair, t, o, hmT, nslots):
                key = (pair, t)
                if key not in ysbs:
                    ysbs[key] = ypool.tile([128, D], f16, tag="ysb",
                                           name=f"ysb{pair}_{t}")
                ysb = ysbs[key]
                yp = ypsum(f"yps{pair}_{t}_{o}", nslots)
                for i in range(ERCH):
                    nc.tensor.matmul(
                        yp, hmT[i][:, 128 * t:128 * t + 128],
                        bwo[:, o, i, :],
                        start=(i == 0), stop=(i == ERCH - 1))
                o0 = 512 * o
                nc.scalar.copy(ysb[:, o0:o0 + 256], yp[:, 0:256])
                nc.vector.tensor_copy(ysb[:, o0 + 256:o0 + 512],
                                      yp[:, 256:512])

            def y_out(pair, t, eng, split=False):
                ysb = ysbs[(pair, t)]
                if split:
                    for o in range(NOCH):
                        e2 = (nc.sync, nc.scalar, nc.gpsimd, nc.scalar)[o]
                        o0 = 512 * o
                        e2.dma_start(out=yo.ap()[pair, t][:, o0:o0 + 512],
                                     in_=ysb[:, o0:o0 + 512])
                else:
                    eng.dma_start(out=yo.ap()[pair, t], in_=ysb)

            # ================= schedule =================
            # PE order: dummies | g0+h0 k-major (dense from the start) |
            # tr0 | g1 | masktr0 | h1 (into y0/y1/lg/tr banks) | y0 chains
            # with pair-1 epilogue PE ops slotted in | y1 chains.
            lg0, hps0 = gating_and_h0()
            issue_b()
            lg_sb0 = lg_copy(lg0)
            eptr0 = tr_logits(0, lg_sb0)
            lg1 = gating1()
            mval0 = softmax_mask(0, eptr0)
            lg_sb1 = lg_copy(lg1)
            mvT0 = mask_transpose(0, eptr0, mval0)
            mexp0 = mask_expand(0, mvT0, nc.sync)
            hps1 = h1_block()
            hm0 = hm_mask(0, hps0, mexp0)

            youtq = (nc.sync, nc.scalar, nc.gpsimd, nc.sync)
            epi1 = {}
            for t in range(NT):
                for o in range(NOCH):
                    y_chain(0, t, o, hm0, 3)
                    # slot pair-1 epilogue PE ops between early chains so
                    # they don't block the chain stream
                    if t == 0 and o == 1:
                        epi1["eptr"] = tr_logits(1, lg_sb1)
                    if t == 0 and o == 2:
                        epi1["mval"] = softmax_mask(1, epi1["eptr"])
                    if t == 1 and o == 0:
                        mvT1 = mask_transpose(1, epi1["eptr"], epi1["mval"])
                        epi1["mexp"] = mask_expand(1, mvT1, nc.gpsimd)
                y_out(0, t, youtq[t])

            hm1 = hm_mask(1, hps1, epi1["mexp"])
            for t in range(NT):
                for o in range(NOCH):
                    y_chain(1, t, o, hm1, 4)
                if t < NT - 1:
                    y_out(1, t, (nc.scalar, nc.gpsimd, nc.sync)[t])
                else:
                    y_out(1, t, None, split=True)

    nc.compile()
    _CACHE["nc"] = nc
    return nc


def _prep_inputs(x, A, Bw, Wg):
    xf = np.asarray(x, dtype=np.float32).reshape(N, D)
    xT = np.ascontiguousarray(xf.T).astype(np.float16)           # [D, N]
    A_t = np.asarray(A, dtype=np.float32).reshape(ER, D).T       # [D, ER]
    af = np.ascontiguousarray(
        A_t.reshape(KD, 128, ER).transpose(1, 0, 2)).astype(np.float16)
    Bwt = (np.asarray(Bw, dtype=np.float32).transpose(0, 2, 1).reshape(ER, D)
           * SCALING)
    bw = np.ascontiguousarray(
        Bwt.reshape(ERCH, 128, NOCH, 512).transpose(1, 2, 0, 3)
    ).astype(np.float16)
    WgT = np.asarray(Wg, dtype=np.float32).T                     # [D, E]
    wg = np.ascontiguousarray(
        WgT.reshape(KD, 128, E).transpose(1, 0, 2)).astype(np.float16)
    in_maps = []
    for c in range(NCORES):
        xc = np.ascontiguousarray(
            xT[:, c * NTOK:(c + 1) * NTOK].reshape(KD, 128, NTOK)
            .transpose(1, 0, 2))
        in_maps.append({"xt": xc, "af": af, "bw": bw, "wg": wg})
    return in_maps


def _run(x, A, Bw, Wg, trace=False):
    nc = _build()
    in_maps = _prep_inputs(x, A, Bw, Wg)
    res = bass_utils.run_bass_kernel_spmd(
        nc, in_maps, core_ids=list(range(NCORES)), trace=trace)
    y = np.concatenate(
        [np.asarray(res.results[c]["yo"], dtype=np.float32).reshape(NTOK, D)
         for c in range(NCORES)], axis=0)
    return y.reshape(B, S, D), res


def kernel(x, A, Bw, Wg):
    y, _ = _run(x, A, Bw, Wg, trace=False)
    return y
